# revision 1
# baseline (speedup 1.0000x reference)
"""Trainium2 Bass kernel for nn_KTM_71339406786898.

Fused dual-input attention block:
  q = wq@(x2+x3)+bq, k = wk@(x2*x3)+bk           (CQ=16 channels)
  energy[i,j] = q[:,i]. k[:,j];  attn = softmax_j
  out{2,3} = v{2,3} @ attn^T;  z{2,3} = gamma*out + x
  h{2,3} = relu(BN(conv3x3(z)));  out = wo@(w2_1@h2 + w3_1@h3 ...)+...

Sharding: data-parallel over batch B=8 across 8 NeuronCores, one batch
per core, params replicated.  Each core computes its [HW,HW] attention
slab tile-by-tile (flash style, j on partitions) so the full attention
matrix never materializes:

  for i-chunk (512 cols):
    for j-tile (128 rows): E_ps = k_jt^T @ q_chunk  (PE, fp32r)
                           E_sb = exp(E_ps)          (ACT)
                           acc += [v2^T|v3^T|1]^T @ E_sb  (PE)
    acc rows 0..63 = unnormalized gamma*out{2,3}; row 64 = softmax denom.
    z = acc[0:64] * recip(acc[64]) + [x2;x3]  -> padded conv buffers

Host-side folds: biases via ones-augmented contraction rows, gamma into
v-weights, BN scale into conv3x3 weights, and wo@w{2,3}_1 into a single
final matmul.
"""

import sys

import ml_dtypes
import numpy as np

for _p in ("/opt/trn_rl_repo", "/root/.axon_site/_ro/trn_rl_repo"):
    if _p not in sys.path:
        sys.path.append(_p)

import concourse.bass as bass
import concourse.mybir as mybir
import concourse.tile as tile
from concourse import bacc
from concourse.bass_utils import run_bass_kernel_spmd

B, C, H, W = 8, 32, 64, 64
CQ = C // 2
HW = H * W
NCORES = 8

IC = 512            # i-chunk (attention query columns per chunk)
NCH = HW // IC      # 8 chunks
JT = 128            # j-tile (attention key rows per tile = partitions)
NJT = HW // JT      # 32 j-tiles
G = 3               # j-tiles per exp granule (ACT op size = G*IC)
USE_ROWTILE = True
PW = W + 2          # padded conv width (66)
PHW = PW * (H + 2)  # padded conv plane (66*66)

F32 = mybir.dt.float32
F32R = mybir.dt.float32r
BF16 = mybir.dt.bfloat16
AF = mybir.ActivationFunctionType
ALU = mybir.AluOpType


def _r(ap):
    """View an f32 AP as float32r for full-rate PE streaming (N>=256)."""
    return ap.bitcast(F32R)


def build_program():
    """Build the single-core Bass/Tile program (SPMD across 8 cores)."""
    nc = bacc.Bacc("TRN2", target_bir_lowering=False, debug=False)

    # --- DRAM I/O (per core) ---
    x2d = nc.dram_tensor("x2", [C, HW], F32R, kind="ExternalInput").ap()
    x3d = nc.dram_tensor("x3", [C, HW], F32R, kind="ExternalInput").ap()
    wqkd = nc.dram_tensor("wqk", [C + 1, 2 * CQ], F32R, kind="ExternalInput").ap()
    wv2d = nc.dram_tensor("wv2a", [C + 1, C], F32R, kind="ExternalInput").ap()
    wv3d = nc.dram_tensor("wv3a", [C + 1, C], F32R, kind="ExternalInput").ap()
    w2Ad = nc.dram_tensor("w2A", [4 * C, C], F32R, kind="ExternalInput").ap()
    w2Bd = nc.dram_tensor("w2B", [4 * C, C], F32R, kind="ExternalInput").ap()
    w2cd = nc.dram_tensor("w2c", [C, C], F32R, kind="ExternalInput").ap()
    w3Ad = nc.dram_tensor("w3A", [4 * C, C], F32R, kind="ExternalInput").ap()
    w3Bd = nc.dram_tensor("w3B", [4 * C, C], F32R, kind="ExternalInput").ap()
    w3cd = nc.dram_tensor("w3c", [C, C], F32R, kind="ExternalInput").ap()
    b2d = nc.dram_tensor("b2", [C, 1], F32, kind="ExternalInput").ap()
    b3d = nc.dram_tensor("b3", [C, 1], F32, kind="ExternalInput").ap()
    wabd = nc.dram_tensor("wab", [2 * C, C], F32R, kind="ExternalInput").ap()
    bfind = nc.dram_tensor("bfin", [C, 1], F32, kind="ExternalInput").ap()
    outd = nc.dram_tensor("out", [C, HW], F32, kind="ExternalOutput").ap()

    with tile.TileContext(nc) as tc:
        _emit(nc, tc, x2d, x3d, wqkd, wv2d, wv3d,
              (w2Ad, w2Bd, w2cd), (w3Ad, w3Bd, w3cd), b2d, b3d,
              wabd, bfind, outd)
    nc.compile()
    return nc


def _emit(nc, tc, x2d, x3d, wqkd, wv2d, wv3d, w2ds, w3ds, b2d, b3d,
          wabd, bfind, outd):
    from contextlib import ExitStack

    ctx = ExitStack()
    with ctx:
        consts = ctx.enter_context(tc.tile_pool(name="consts", bufs=1))
        xa = ctx.enter_context(tc.tile_pool(name="xa", bufs=1))
        sc = ctx.enter_context(tc.tile_pool(name="scratch", bufs=2))
        stk = ctx.enter_context(tc.tile_pool(name="stk", bufs=1))
        qk = ctx.enter_context(tc.tile_pool(name="qk", bufs=1))
        vs = ctx.enter_context(tc.tile_pool(name="vs", bufs=1))
        es = ctx.enter_context(tc.tile_pool(name="es", bufs=3))
        zs = ctx.enter_context(tc.tile_pool(name="zs", bufs=2))
        outp = ctx.enter_context(tc.tile_pool(name="outp", bufs=2))
        psum = ctx.enter_context(tc.tile_pool(name="psum", bufs=2, space="PSUM"))
        accp = ctx.enter_context(tc.tile_pool(name="accp", bufs=1, space="PSUM"))
        convp = ctx.enter_context(tc.tile_pool(name="convp", bufs=1, space="PSUM"))

        # --- load constants ---
        wqk_sb = consts.tile([C + 1, 2 * CQ], F32R, tag="wqk")
        nc.sync.dma_start(out=wqk_sb[:], in_=wqkd)
        wv2_sb = consts.tile([C + 1, C], F32R, tag="wv2")
        nc.sync.dma_start(out=wv2_sb[:], in_=wv2d)
        wv3_sb = consts.tile([C + 1, C], F32R, tag="wv3")
        nc.sync.dma_start(out=wv3_sb[:], in_=wv3d)
        w2sb = []
        for nm, d in zip(("w2A", "w2B", "w2c"), w2ds):
            t = consts.tile(list(d.shape), F32R, tag=nm)
            nc.sync.dma_start(out=t[:], in_=d)
            w2sb.append(t)
        w3sb = []
        for nm, d in zip(("w3A", "w3B", "w3c"), w3ds):
            t = consts.tile(list(d.shape), F32R, tag=nm)
            nc.sync.dma_start(out=t[:], in_=d)
            w3sb.append(t)
        b2_sb = consts.tile([C, 1], F32, tag="b2")
        nc.sync.dma_start(out=b2_sb[:], in_=b2d)
        b3_sb = consts.tile([C, 1], F32, tag="b3")
        nc.sync.dma_start(out=b3_sb[:], in_=b3d)
        wab_sb = consts.tile([2 * C, C], F32R, tag="wab")
        nc.sync.dma_start(out=wab_sb[:], in_=wabd)
        bfin_sb = consts.tile([C, 1], F32, tag="bfin")
        nc.sync.dma_start(out=bfin_sb[:], in_=bfind)

        # --- inputs with ones-augmented row (bias via matmul) ---
        x2a = xa.tile([C + 1, HW], F32R, tag="x2a")
        nc.sync.dma_start(out=x2a[0:C, :], in_=x2d)
        nc.vector.memset(x2a[C:C + 1, :].bitcast(F32), 1.0)
        x3a = xa.tile([C + 1, HW], F32R, tag="x3a")
        nc.sync.dma_start(out=x3a[0:C, :], in_=x3d)
        nc.vector.memset(x3a[C:C + 1, :].bitcast(F32), 1.0)

        xsum = sc.tile([C + 1, PHW], F32R, tag="sc33")
        nc.vector.tensor_add(xsum[0:C, 0:HW], x2a[0:C, :], x3a[0:C, :])
        nc.vector.memset(xsum[C:C + 1, 0:HW].bitcast(F32), 1.0)
        xmul = sc.tile([C + 1, PHW], F32R, tag="sc33")
        nc.vector.tensor_mul(xmul[0:C, 0:HW], x2a[0:C, :], x3a[0:C, :])
        nc.vector.memset(xmul[C:C + 1, 0:HW].bitcast(F32), 1.0)

        # --- q / k projections, replicated at partition offsets 0/32/64
        # for row-tiled (tile_position) energy matmuls ---
        q_sb = qk.tile([2 * C + CQ, HW], BF16, tag="q")
        k_sb = qk.tile([2 * C + CQ, HW], BF16, tag="k")
        QKC = 1536  # psum chunk (3 banks)
        for (src, dst, col0) in ((xsum, q_sb, 0), (xmul, k_sb, CQ)):
            off = 0
            while off < HW:
                n = min(QKC, HW - off)
                p = psum.tile([CQ, QKC], F32, tag="big")
                for s in range(0, n, 512):
                    nc.tensor.matmul(
                        p[:, s:s + 512],
                        _r(wqk_sb[:, col0:col0 + CQ]),
                        _r(src[:, off + s:off + s + 512]),
                        start=True, stop=True,
                    )
                nc.vector.tensor_copy(out=dst[0:CQ, off:off + n], in_=p[:, 0:n])
                off += n
            for rg in (1, 2):
                nc.sync.dma_start(out=dst[32 * rg:32 * rg + CQ, :],
                                  in_=dst[0:CQ, :])

        # --- v stack: vstack[j, jt, c] ; col 64 = ones (softmax denom) ---
        vstack = vs.tile([JT, NJT, 2 * C + 1], BF16, tag="vstack")
        nc.vector.memset(vstack[:, :, 2 * C:2 * C + 1], 1.0)
        for jt in range(NJT):
            vp = psum.tile([JT, 2 * C], F32, tag="big")
            nc.tensor.matmul(
                vp[:, 0:C],
                _r(x2a[:, jt * JT:(jt + 1) * JT]),
                _r(wv2_sb[:]),
                start=True, stop=True,
            )
            nc.tensor.matmul(
                vp[:, C:2 * C],
                _r(x3a[:, jt * JT:(jt + 1) * JT]),
                _r(wv3_sb[:]),
                start=True, stop=True,
            )
            nc.vector.tensor_copy(out=vstack[:, jt, 0:2 * C], in_=vp[:])

        # --- padded conv inputs (zero borders) ---
        z2p = sc.tile([C + 1, PHW], F32R, tag="sc33")
        nc.vector.memset(z2p[0:C, :].bitcast(F32), 0.0)
        z3p = sc.tile([C + 1, PHW], F32R, tag="sc33")
        nc.vector.memset(z3p[0:C, :].bitcast(F32), 0.0)
        z2p3 = z2p[0:C, :].rearrange("p (h w) -> p h w", h=H + 2, w=PW)
        z3p3 = z3p[0:C, :].rearrange("p (h w) -> p h w", h=H + 2, w=PW)

        # persistent K-pack stacks (4 shifted tap copies along partitions)
        stkA2 = stk.tile([JT, PHW], F32R, tag="stkA2")
        stkB2 = stk.tile([JT, PHW], F32R, tag="stkB2")
        stkA3 = stk.tile([JT, PHW], F32R, tag="stkA3")
        stkB3 = stk.tile([JT, PHW], F32R, tag="stkB3")
        stk3d = {}
        for nm, t in (("A2", stkA2), ("B2", stkB2), ("A3", stkA3), ("B3", stkB3)):
            stk3d[nm] = t.rearrange("p (h w) -> p h w", h=H + 2, w=PW)

        RPC = IC // W  # spatial rows per chunk (8)
        SEG = RPC * PW + W  # stack copy length per chunk (592)

        def conv_chunk(n):
            """conv3x3 + relu + fused final 1x1 for output chunk n.
            Requires z-chunks n and n+1 to be written."""
            p0 = PW * RPC * n
            for (zp, nmA, nmB) in ((z2p, "A2", "B2"), (z3p, "A3", "B3")):
                stA = stkA2 if nmA == "A2" else stkA3
                stB = stkB2 if nmB == "B2" else stkB3
                ln = min(SEG, PHW - p0 - 2 * PW - 2)
                for a in range(4):
                    offA = (a // 3) * PW + (a % 3)
                    nc.sync.dma_start(
                        out=stA[32 * a:32 * a + C, p0:p0 + ln],
                        in_=zp[0:C, p0 + offA:p0 + offA + ln])
                    tb = a + 4
                    offB = (tb // 3) * PW + (tb % 3)
                    nc.gpsimd.dma_start(
                        out=stB[32 * a:32 * a + C, p0:p0 + ln],
                        in_=zp[0:C, p0 + offB:p0 + offB + ln])
            r0 = RPC * n
            rstk = outp.tile([2 * C, IC], F32R, tag="rstk")
            for (s3A, s3B, zp3, ws, bb, row0) in (
                    (stk3d["A2"], stk3d["B2"], z2p3, w2sb, b2_sb, 0),
                    (stk3d["A3"], stk3d["B3"], z3p3, w3sb, b3_sb, C)):
                cp = convp.tile([C, IC], F32, tag="cv")
                nc.tensor.matmul(
                    cp[:], ws[0][:], s3A[:, r0:r0 + RPC, 0:W],
                    start=True, stop=False)
                nc.tensor.matmul(
                    cp[:], ws[1][:], s3B[:, r0:r0 + RPC, 0:W],
                    start=False, stop=False)
                nc.tensor.matmul(
                    cp[:], ws[2][:], zp3[:, 2 + r0:2 + r0 + RPC, 2:2 + W],
                    start=False, stop=True)
                nc.vector.tensor_scalar(
                    out=rstk[row0:row0 + C, :],
                    in0=cp[:],
                    scalar1=bb[:, 0:1], scalar2=0.0,
                    op0=ALU.add, op1=ALU.max,
                )
            op = convp.tile([C, IC], F32, tag="cv")
            nc.tensor.matmul(
                op[:], _r(wab_sb[:]), _r(rstk[:]), start=True, stop=True)
            ob = outp.tile([C, IC], F32, tag="ob")
            nc.vector.tensor_scalar(
                out=ob[:], in0=op[:],
                scalar1=bfin_sb[:, 0:1], scalar2=None, op0=ALU.add,
            )
            nc.sync.dma_start(out=outd[:, n * IC:(n + 1) * IC], in_=ob[:])

        # --- main attention loop, conv pipelined one chunk behind ---
        for ic in range(NCH):
            i0 = ic * IC
            acc = accp.tile([2 * C + 1, IC], F32, tag="acc")
            jt = 0
            first = True
            while jt < NJT:
                g = min(G, NJT - jt)
                e_ps = psum.tile([JT, G * IC], F32, tag="big")
                for t in range(g):
                    rt = t if USE_ROWTILE else 0
                    nc.tensor.matmul(
                        e_ps[:, t * IC:(t + 1) * IC],
                        k_sb[32 * rt:32 * rt + CQ,
                             (jt + t) * JT:(jt + t + 1) * JT],
                        q_sb[32 * rt:32 * rt + CQ, i0:i0 + IC],
                        start=True, stop=True,
                        tile_position=(32 * rt, 0) if USE_ROWTILE else None,
                    )
                e_sb = es.tile([JT, G * IC], BF16, tag="esb")
                nc.scalar.activation(
                    e_sb[:, 0:g * IC], e_ps[:, 0:g * IC], AF.Exp)
                for t in range(g):
                    nc.tensor.matmul(
                        acc[:],
                        vstack[:, jt + t, :],
                        e_sb[:, t * IC:(t + 1) * IC],
                        start=first, stop=(jt + t == NJT - 1),
                    )
                    first = False
                jt += g

            # normalize + residual -> padded conv buffers.  Pull the
            # unnormalized rows out of psum first (short ops) so the
            # accumulator bank frees before the slow reciprocal runs.
            s_sb = zs.tile([1, IC], F32, tag="s_sb")
            nc.vector.tensor_copy(out=s_sb[:], in_=acc[2 * C:2 * C + 1, :])
            u2 = zs.tile([C, IC], F32, tag="u2")
            nc.vector.tensor_copy(out=u2[:], in_=acc[0:C, :])
            u3 = zs.tile([C, IC], F32, tag="u3")
            nc.vector.tensor_copy(out=u3[:], in_=acc[C:2 * C, :])
            r_sb = zs.tile([1, IC], F32, tag="r_sb")
            nc.vector.reciprocal(r_sb[:], s_sb[:])
            rbc = zs.tile([C, IC], F32, tag="rbc")
            nc.gpsimd.partition_broadcast(rbc[:], r_sb[:])
            z2t = zs.tile([C, IC], F32, tag="z2t")
            nc.vector.tensor_mul(z2t[:], u2[:], rbc[0:C, :])
            z3t = zs.tile([C, IC], F32, tag="z3t")
            nc.vector.tensor_mul(z3t[:], u3[:], rbc[0:C, :])
            r0 = RPC * ic
            nc.vector.tensor_add(
                z2p3[:, 1 + r0:1 + r0 + RPC, 1:1 + W],
                z2t[:].rearrange("p (a b) -> p a b", a=RPC, b=W),
                x2a[0:C, i0:i0 + IC].rearrange("p (a b) -> p a b", a=RPC, b=W),
            )
            nc.vector.tensor_add(
                z3p3[:, 1 + r0:1 + r0 + RPC, 1:1 + W],
                z3t[:].rearrange("p (a b) -> p a b", a=RPC, b=W),
                x3a[0:C, i0:i0 + IC].rearrange("p (a b) -> p a b", a=RPC, b=W),
            )
            if ic >= 1:
                conv_chunk(ic - 1)
        conv_chunk(NCH - 1)


def prepare_params(wq, bq, wk, bk, wv2, bv2, wv3, bv3, gamma2, gamma3,
                   w2_3, bn2_s, bn2_b, w2_1, b2_1,
                   w3_3, bn3_s, bn3_b, w3_1, b3_1, wo, bo):
    """Fold params into the device layouts (see build_program docstring)."""
    f = np.float32
    wq, bq, wk, bk = (np.asarray(a, f) for a in (wq, bq, wk, bk))
    wv2, bv2, wv3, bv3 = (np.asarray(a, f) for a in (wv2, bv2, wv3, bv3))
    w2_3, bn2_s, bn2_b = (np.asarray(a, f) for a in (w2_3, bn2_s, bn2_b))
    w3_3, bn3_s, bn3_b = (np.asarray(a, f) for a in (w3_3, bn3_s, bn3_b))
    w2_1, b2_1, w3_1, b3_1 = (np.asarray(a, f) for a in (w2_1, b2_1, w3_1, b3_1))
    wo, bo = np.asarray(wo, f), np.asarray(bo, f)
    g2 = f(np.asarray(gamma2).reshape(-1)[0])
    g3 = f(np.asarray(gamma3).reshape(-1)[0])

    wqk = np.zeros((C + 1, 2 * CQ), f)
    wqk[:C, 0:CQ] = wq.T
    wqk[C, 0:CQ] = bq
    wqk[:C, CQ:2 * CQ] = wk.T
    wqk[C, CQ:2 * CQ] = bk

    wv2a = np.zeros((C + 1, C), f)
    wv2a[:C] = wv2.T * g2
    wv2a[C] = bv2 * g2
    wv3a = np.zeros((C + 1, C), f)
    wv3a[:C] = wv3.T * g3
    wv3a[C] = bv3 * g3

    bf = ml_dtypes.bfloat16

    def conv_stacks(w3x3, bn_s):
        ws = w3x3 * bn_s[:, None, None, None]  # [o, ci, dy, dx]
        A = np.zeros((4 * C, C), f)
        Bm = np.zeros((4 * C, C), f)
        for a in range(4):
            A[32 * a:32 * a + C] = ws[:, :, a // 3, a % 3].T
            tb = a + 4
            Bm[32 * a:32 * a + C] = ws[:, :, tb // 3, tb % 3].T
        cm = ws[:, :, 2, 2].T.copy()
        return A, Bm, cm

    w2A, w2B, w2c = conv_stacks(w2_3, bn2_s)
    w3A, w3B, w3c = conv_stacks(w3_3, bn3_s)

    wab = np.zeros((2 * C, C), f)
    wab[:C] = (wo @ w2_1).T
    wab[C:] = (wo @ w3_1).T
    bfin = (wo @ (b2_1 + b3_1) + bo).astype(f)

    return {
        "wqk": wqk, "wv2a": wv2a, "wv3a": wv3a,
        "w2A": w2A, "w2B": w2B, "w2c": w2c,
        "w3A": w3A, "w3B": w3B, "w3c": w3c,
        "b2": bn2_b.reshape(C, 1).astype(f),
        "b3": bn3_b.reshape(C, 1).astype(f),
        "wab": wab, "bfin": bfin.reshape(C, 1),
    }


_CACHED = {}


def _get_program():
    if "nc" not in _CACHED:
        _CACHED["nc"] = build_program()
    return _CACHED["nc"]


def make_in_maps(x2, x3, params):
    x2 = np.ascontiguousarray(np.asarray(x2, np.float32).reshape(B, C, HW))
    x3 = np.ascontiguousarray(np.asarray(x3, np.float32).reshape(B, C, HW))
    return [
        {"x2": x2[b], "x3": x3[b], **params}
        for b in range(NCORES)
    ]


def kernel(x2, x3, **kw):
    params = prepare_params(**kw)
    nc = _get_program()
    in_maps = make_in_maps(x2, x3, params)
    res = run_bass_kernel_spmd(nc, in_maps, list(range(NCORES)))
    out = np.stack([res.results[b]["out"].reshape(C, H, W)
                    for b in range(NCORES)])
    return out.astype(np.float32)


def _ensure_ntff_hook():
    """The agent image's antenv lacks axon_hooks; register the ctypes
    NTFF profile hook ourselves (mirrors trn_agent_boot.trn_boot)."""
    import contextlib
    import ctypes
    import types

    if "antenv.axon_hooks" in sys.modules:
        return
    so_path = "/opt/axon/libaxon_pjrt.so"
    lib = ctypes.CDLL(so_path)
    lib.axon_start_nrt_profile.argtypes = [
        ctypes.POINTER(ctypes.c_int64), ctypes.c_size_t]
    lib.axon_start_nrt_profile.restype = ctypes.c_int64
    lib.axon_stop_nrt_profile.argtypes = [ctypes.c_char_p]
    lib.axon_stop_nrt_profile.restype = ctypes.c_int64

    @contextlib.contextmanager
    def _hook(output_dir, device_ids):
        import jax
        jax.devices()
        if device_ids:
            ids = (ctypes.c_int64 * len(device_ids))(*device_ids)
            rc = lib.axon_start_nrt_profile(ids, len(device_ids))
        else:
            rc = lib.axon_start_nrt_profile(None, 0)
        if rc != 0:
            raise RuntimeError(f"axon_start_nrt_profile rc={rc}")
        try:
            yield
        finally:
            n = lib.axon_stop_nrt_profile(str(output_dir).encode())
            if n < 0:
                raise RuntimeError(f"axon_stop_nrt_profile rc={n}")
            if n == 0:
                print("WARNING: NTFF capture wrote 0 files")

    mod = types.ModuleType("antenv.axon_hooks")
    mod.get_axon_ntff_profile_hook = lambda: _hook
    mod.set_axon_ntff_profile_hook = lambda h: None
    sys.modules["antenv.axon_hooks"] = mod


def run_traced(x2, x3, trace_cores=None, **kw):
    """Like kernel() but returns (out, BassKernelResults) with profiling."""
    _ensure_ntff_hook()
    params = prepare_params(**kw)
    nc = _get_program()
    in_maps = make_in_maps(x2, x3, params)
    res = run_bass_kernel_spmd(nc, in_maps, list(range(NCORES)),
                               trace=True, trace_cores=trace_cores)
    out = np.stack([res.results[b]["out"].reshape(C, H, W)
                    for b in range(NCORES)])
    return out.astype(np.float32), res

